# revision 13
# baseline (speedup 1.0000x reference)
"""MHSA kernel for 8 Trainium2 NeuronCores — Bass/Tile implementation.

Distribution (per sharding hint): data-parallel over batch (4) x
tensor-parallel over head-groups (2 groups of 8 heads) = 8 shards.

Per-core math (all layouts transposed so no on-device transposes needed):
  inputs:  XT [C, N] = x[b].T          (bf16)
           WqT/WkT/WvT [C, DPG]        (bf16, pre-transposed slices of w_qkv)
           WoT [DPG, C]                (bf16, pre-transposed slice of w_out)
  QT = (WqT.T @ XT).T stored as [DPG, N]   (d on partitions)
  KT likewise; V natural [N, DPG] augmented with a ones column per head.
  For each head h: ST = K_h Q_h^T  -> [n2, n1] tiles in PSUM,
  exp on ScalarE (scale folded), AV via matmul with augmented V ->
  O'T [65, n1] where row 64 = softmax denominators (ones-column trick).
  Normalize O^T rows, project Y = (O^T).T @ WoT in PSUM, DMA out fp32.

Head pairs (2t, 2t+1) run their K=64 score matmuls concurrently via
PE row-tiling (partitions 0-63 / 64-127 -> tile_position auto-derived).

Host: shard prep (transpose/cast/concat) + partial-sum gather + bias.
"""
import sys

if "/opt/trn_rl_repo" not in sys.path:
    sys.path.insert(0, "/opt/trn_rl_repo")

from contextlib import ExitStack

import numpy as np
import ml_dtypes

import jax
import jax.numpy as jnp
from jax.sharding import Mesh, NamedSharding, PartitionSpec as P
from jax.experimental.shard_map import shard_map

import concourse.bass as bass
import concourse.mybir as mybir
import concourse.tile as tile
from concourse.bass2jax import bass_jit, fast_dispatch_compile

B, N, C, H = 4, 2048, 1024, 16
HD = C // H            # 64
NCORES = 8
TP = 2                 # head groups
HPG = H // TP          # 8 heads per group
DPG = HPG * HD         # 512 dims per group

BF16 = mybir.dt.bfloat16
F32 = mybir.dt.float32
SCALE = float(HD) ** -0.5

_cache = {}


def _mhsa_body(ctx, tc, xt, wq, wk, wv, wo, y,
               nseq=N, cin=C, hpg=HPG, hd=HD, jb=1024):
    """Emit the per-core MHSA program.

    xt [cin, nseq] bf16, wq/wk/wv [cin, dpg] bf16, wo [dpg, cin_out] bf16,
    y [nseq, cin_out] f32.  dpg = hpg*hd, heads processed in pairs.
    """
    nc = tc.nc
    dpg = hpg * hd
    cout = wo.shape[1]
    NCH = cin // 128          # contraction chunks for projections
    ND = dpg // 128           # d-chunks == head pairs
    NN = nseq // 128          # n2 chunks
    FB = min(512, jb)         # matmul free-dim block
    NJ = nseq // jb           # n1 blocks
    NU = jb // FB
    FBO = min(512, cout)      # output projection free block
    NCB = cout // FBO
    EXP = mybir.ActivationFunctionType.Exp

    singles = ctx.enter_context(tc.tile_pool(name="singles", bufs=1))
    xts = singles.tile([128, NCH, nseq], BF16)
    wqs = singles.tile([128, NCH, dpg], BF16)
    wks = singles.tile([128, NCH, dpg], BF16)
    wvs = singles.tile([128, NCH, dpg], BF16)
    wos = singles.tile([128, ND, cout], BF16)
    vaug = singles.tile([128, NN, hpg, hd + 1], BF16)
    qts = singles.tile([128, ND, nseq], BF16)
    kts = singles.tile([128, ND, nseq], BF16)
    ot = singles.tile([128, ND, nseq], BF16)

    for c in range(NCH):
        nc.sync.dma_start(out=xts[:, c, :], in_=xt[c * 128:(c + 1) * 128, :])
        nc.sync.dma_start(out=wqs[:, c, :], in_=wq[c * 128:(c + 1) * 128, :])
        nc.sync.dma_start(out=wks[:, c, :], in_=wk[c * 128:(c + 1) * 128, :])
        nc.sync.dma_start(out=wvs[:, c, :], in_=wv[c * 128:(c + 1) * 128, :])
    for t in range(ND):
        nc.sync.dma_start(out=wos[:, t, :], in_=wo[t * 128:(t + 1) * 128, :])

    nc.gpsimd.memset(vaug, 1.0)

    # ---- projections: V (natural layout, augmented), QT/KT (transposed) ----
    with tc.tile_pool(name="ppool", bufs=2, space="PSUM") as ppool:
        for m in range(NN):
            pv = ppool.tile([128, dpg], F32, tag="pv")
            for c in range(NCH):
                nc.tensor.matmul(pv, lhsT=xts[:, c, m * 128:(m + 1) * 128],
                                 rhs=wvs[:, c, :],
                                 start=(c == 0), stop=(c == NCH - 1))
            nc.vector.tensor_copy(vaug[:, m, :, 0:hd],
                                  pv.rearrange("p (h d) -> p h d", d=hd))
        for t in range(ND):
            for j in range(nseq // FB):
                pq = ppool.tile([128, FB], F32, tag="pq")
                pk = ppool.tile([128, FB], F32, tag="pk")
                for c in range(NCH):
                    nc.tensor.matmul(pq, lhsT=wqs[:, c, t * 128:(t + 1) * 128],
                                     rhs=xts[:, c, j * FB:(j + 1) * FB],
                                     start=(c == 0), stop=(c == NCH - 1))
                for c in range(NCH):
                    nc.tensor.matmul(pk, lhsT=wks[:, c, t * 128:(t + 1) * 128],
                                     rhs=xts[:, c, j * FB:(j + 1) * FB],
                                     start=(c == 0), stop=(c == NCH - 1))
                nc.vector.tensor_copy(qts[:, t, j * FB:(j + 1) * FB], pq)
                nc.vector.tensor_copy(kts[:, t, j * FB:(j + 1) * FB], pk)

    # ---- attention (head pairs A/B, exp batched over jb-wide n1 blocks) ----
    with tc.tile_pool(name="atp", bufs=1, space="PSUM") as atp, \
         tc.tile_pool(name="opp", bufs=1, space="PSUM") as opp, \
         tc.tile_pool(name="apool", bufs=3) as apool, \
         tc.tile_pool(name="ospool", bufs=2) as ospool, \
         tc.tile_pool(name="rpool", bufs=2) as rpool:
        for t in range(ND):
            for jj in range(NJ):
                oA = opp.tile([hd + 1, jb], F32, tag="oA")
                oB = opp.tile([hd + 1, jb], F32, tag="oB")
                for i in range(NN):
                    stA = atp.tile([128, jb], F32, tag="stA")
                    stB = atp.tile([128, jb], F32, tag="stB")
                    for u in range(NU):
                        fs = slice(u * FB, (u + 1) * FB)
                        qs = slice(jj * jb + u * FB, jj * jb + (u + 1) * FB)
                        nc.tensor.matmul(
                            stA[:, fs], lhsT=kts[0:hd, t, i * 128:(i + 1) * 128],
                            rhs=qts[0:hd, t, qs], start=True, stop=True)
                        nc.tensor.matmul(
                            stB[:, fs], lhsT=kts[hd:2 * hd, t, i * 128:(i + 1) * 128],
                            rhs=qts[hd:2 * hd, t, qs], start=True, stop=True)
                    aA = apool.tile([128, jb], BF16, tag="aA")
                    aB = apool.tile([128, jb], BF16, tag="aB")
                    nc.scalar.activation(aA, stA, EXP, scale=SCALE)
                    nc.scalar.activation(aB, stB, EXP, scale=SCALE)
                    for u in range(NU):
                        fs = slice(u * FB, (u + 1) * FB)
                        nc.tensor.matmul(oA[:, fs], lhsT=vaug[:, i, 2 * t, :],
                                         rhs=aA[:, fs],
                                         start=(i == 0), stop=(i == NN - 1))
                        nc.tensor.matmul(oB[:, fs], lhsT=vaug[:, i, 2 * t + 1, :],
                                         rhs=aB[:, fs],
                                         start=(i == 0), stop=(i == NN - 1))
                # drain PSUM quickly, then normalize out of SBUF
                oAs = ospool.tile([hd + 1, jb], F32, tag="oAs")
                oBs = ospool.tile([hd + 1, jb], F32, tag="oBs")
                nc.vector.tensor_copy(oAs, oA)
                nc.vector.tensor_copy(oBs, oB)
                js = slice(jj * jb, (jj + 1) * jb)
                for b, os_ in ((0, oAs), (1, oBs)):
                    r = rpool.tile([1, jb], F32, tag=f"r{b}")
                    R = rpool.tile([hd, jb], F32, tag=f"R{b}")
                    nc.vector.reciprocal(r, os_[hd:hd + 1, :])
                    nc.gpsimd.partition_broadcast(R, r)
                    nc.vector.tensor_mul(ot[b * hd:(b + 1) * hd, t, js],
                                         os_[0:hd, :], R)

    # ---- output projection ----
    with tc.tile_pool(name="ypp", bufs=2, space="PSUM") as ypp, \
         tc.tile_pool(name="ysbp", bufs=3) as ysbp:
        for m in range(NN):
            ys = ysbp.tile([128, cout], y.dtype, tag="ys")
            for cb in range(NCB):
                py = ypp.tile([128, FBO], F32, tag="py")
                for t in range(ND):
                    nc.tensor.matmul(py, lhsT=ot[:, t, m * 128:(m + 1) * 128],
                                     rhs=wos[:, t, cb * FBO:(cb + 1) * FBO],
                                     start=(t == 0), stop=(t == ND - 1))
                nc.vector.tensor_copy(ys[:, cb * FBO:(cb + 1) * FBO], py)
            nc.sync.dma_start(out=y[m * 128:(m + 1) * 128, :], in_=ys)


def _build(nc, xt, wq, wk, wv, wo):
    y = nc.dram_tensor("y", [N, C], BF16, kind="ExternalOutput")
    with tile.TileContext(nc) as tc:
        with ExitStack() as ctx:
            _mhsa_body(ctx, tc, xt[:], wq[:], wk[:], wv[:], wo[:], y[:])
    return y


def _get_compiled():
    if "fn" in _cache:
        return _cache["fn"], _cache["mesh"]
    devs = jax.devices()[:NCORES]
    mesh = Mesh(np.asarray(devs), ("core",))
    sh = NamedSharding(mesh, P("core"))
    bf = ml_dtypes.bfloat16
    structs = tuple(
        jax.ShapeDtypeStruct((NCORES * d0, d1), bf, sharding=sh)
        for d0, d1 in ((C, N), (C, DPG), (C, DPG), (C, DPG), (DPG, C))
    )

    def compile_fn():
        bj = bass_jit(_build)
        f = jax.jit(
            shard_map(
                lambda *a: bj(*a),
                mesh=mesh,
                in_specs=(P("core"),) * 5,
                out_specs=P("core"),
                check_rep=False,
            )
        )
        return f.lower(*structs).compile()

    fn = fast_dispatch_compile(compile_fn)
    _cache["fn"] = fn
    _cache["mesh"] = mesh
    _cache["sharding"] = sh
    return fn, mesh


def _device_put(shards):
    """Place shard arrays with the correct core sharding (avoids a costly
    per-call reshard from device 0)."""
    _get_compiled()
    sh = _cache["sharding"]
    return tuple(jax.device_put(s, sh) for s in shards)


def _make_shards(x, w_qkv, w_out):
    """Per-core inputs, concatenated along axis 0 (core c -> b=c//2, t=c%2)."""
    bf = ml_dtypes.bfloat16
    w_q, w_k, w_v = w_qkv[0:C], w_qkv[C:2 * C], w_qkv[2 * C:3 * C]
    xts, wqs, wks, wvs, wos = [], [], [], [], []
    for core in range(NCORES):
        b, t = divmod(core, TP)
        sl = slice(t * DPG, (t + 1) * DPG)
        xts.append(np.ascontiguousarray(x[b].T).astype(bf))          # [C, N]
        wqs.append(np.ascontiguousarray(w_q[sl].T).astype(bf))       # [C, DPG]
        wks.append(np.ascontiguousarray(w_k[sl].T).astype(bf))
        wvs.append(np.ascontiguousarray(w_v[sl].T).astype(bf))
        wos.append(np.ascontiguousarray(w_out[:, sl].T).astype(bf))  # [DPG, C]
    return (
        np.concatenate(xts, axis=0),
        np.concatenate(wqs, axis=0),
        np.concatenate(wks, axis=0),
        np.concatenate(wvs, axis=0),
        np.concatenate(wos, axis=0),
    )


def kernel(x, w_qkv, w_out, b_out):
    x = np.asarray(x, dtype=np.float32)
    w_qkv = np.asarray(w_qkv, dtype=np.float32)
    w_out = np.asarray(w_out, dtype=np.float32)
    b_out = np.asarray(b_out, dtype=np.float32)

    fn, _ = _get_compiled()
    shards = _device_put(_make_shards(x, w_qkv, w_out))
    out = np.asarray(jax.block_until_ready(fn(*shards)))   # [8*N, C]
    parts = out.reshape(NCORES, N, C).astype(np.float32)
    res = np.empty((B, N, C), dtype=np.float32)
    for b in range(B):
        res[b] = parts[TP * b] + parts[TP * b + 1] + b_out[None, :]
    return res


if __name__ == "__main__":
    rng = np.random.default_rng(0)
    x = rng.standard_normal((B, N, C), dtype=np.float32)
    w_qkv = rng.standard_normal((3 * C, C), dtype=np.float32) * C ** -0.5
    w_out = rng.standard_normal((C, C), dtype=np.float32) * C ** -0.5
    b_out = rng.standard_normal(C, dtype=np.float32) * 0.01
    o = kernel(x=x, w_qkv=w_qkv, w_out=w_out, b_out=b_out)
    print("kernel ran, out shape", o.shape)


# revision 14
# speedup vs baseline: 1.2317x; 1.2317x over previous
"""MHSA kernel for 8 Trainium2 NeuronCores — Bass/Tile implementation.

Distribution (per sharding hint): data-parallel over batch (4) x
tensor-parallel over head-groups (2 groups of 8 heads) = 8 shards.

Per-core math (all layouts transposed so no on-device transposes needed):
  inputs:  XT [C, N] = x[b].T          (bf16)
           WqT/WkT/WvT [C, DPG]        (bf16, pre-transposed slices of w_qkv)
           WoT [DPG, C]                (bf16, pre-transposed slice of w_out)
  QT = (WqT.T @ XT).T stored as [DPG, N]   (d on partitions)
  KT likewise; V natural [N, DPG] augmented with a ones column per head.
  For each head h: ST = K_h Q_h^T  -> [n2, n1] tiles in PSUM,
  exp on ScalarE (scale folded), AV via matmul with augmented V ->
  O'T [65, n1] where row 64 = softmax denominators (ones-column trick).
  Normalize O^T rows, project Y = (O^T).T @ WoT in PSUM, DMA out bf16
  (host upcasts, sums the two TP partials per batch, adds bias in fp32).

Head pairs (2t, 2t+1) run their K=64 score matmuls concurrently via
PE row-tiling (partitions 0-63 / 64-127 -> tile_position auto-derived).

Host: shard prep (transpose/cast/concat) + partial-sum gather + bias.
"""
import sys

if "/opt/trn_rl_repo" not in sys.path:
    sys.path.insert(0, "/opt/trn_rl_repo")

from contextlib import ExitStack

import numpy as np
import ml_dtypes

import jax
import jax.numpy as jnp
from jax.sharding import Mesh, NamedSharding, PartitionSpec as P
from jax.experimental.shard_map import shard_map

import concourse.bass as bass
import concourse.mybir as mybir
import concourse.tile as tile
from concourse.bass2jax import bass_jit, fast_dispatch_compile

B, N, C, H = 4, 2048, 1024, 16
HD = C // H            # 64
NCORES = 8
TP = 2                 # head groups
HPG = H // TP          # 8 heads per group
DPG = HPG * HD         # 512 dims per group

BF16 = mybir.dt.bfloat16
F32 = mybir.dt.float32
SCALE = float(HD) ** -0.5

_cache = {}


def _mhsa_body(ctx, tc, xt, wq, wk, wv, wo, y,
               nseq=N, cin=C, hpg=HPG, hd=HD, jb=1024):
    """Emit the per-core MHSA program.

    xt [cin, nseq] bf16, wq/wk/wv [cin, dpg] bf16, wo [dpg, cin_out] bf16,
    y [nseq, cin_out] f32.  dpg = hpg*hd, heads processed in pairs.
    """
    nc = tc.nc
    dpg = hpg * hd
    cout = wo.shape[1]
    NCH = cin // 128          # contraction chunks for projections
    ND = dpg // 128           # d-chunks == head pairs
    NN = nseq // 128          # n2 chunks
    FB = min(512, jb)         # matmul free-dim block
    NJ = nseq // jb           # n1 blocks
    NU = jb // FB
    FBO = min(512, cout)      # output projection free block
    NCB = cout // FBO
    EXP = mybir.ActivationFunctionType.Exp

    singles = ctx.enter_context(tc.tile_pool(name="singles", bufs=1))
    xts = singles.tile([128, NCH, nseq], BF16)
    wqs = singles.tile([128, NCH, dpg], BF16)
    wks = singles.tile([128, NCH, dpg], BF16)
    wvs = singles.tile([128, NCH, dpg], BF16)
    wos = singles.tile([128, ND, cout], BF16)
    vaug = singles.tile([128, NN, hpg, hd + 1], BF16)
    qts = singles.tile([128, ND, nseq], BF16)
    kts = singles.tile([128, ND, nseq], BF16)
    ot = singles.tile([128, ND, nseq], BF16)

    for c in range(NCH):
        nc.sync.dma_start(out=xts[:, c, :], in_=xt[c * 128:(c + 1) * 128, :])
        nc.sync.dma_start(out=wqs[:, c, :], in_=wq[c * 128:(c + 1) * 128, :])
        nc.sync.dma_start(out=wks[:, c, :], in_=wk[c * 128:(c + 1) * 128, :])
        nc.sync.dma_start(out=wvs[:, c, :], in_=wv[c * 128:(c + 1) * 128, :])
    for t in range(ND):
        nc.sync.dma_start(out=wos[:, t, :], in_=wo[t * 128:(t + 1) * 128, :])

    nc.gpsimd.memset(vaug, 1.0)

    # ---- projections: V (natural layout, augmented), QT/KT (transposed) ----
    with tc.tile_pool(name="ppool", bufs=2, space="PSUM") as ppool:
        for m in range(NN):
            pv = ppool.tile([128, dpg], F32, tag="pv")
            for c in range(NCH):
                nc.tensor.matmul(pv, lhsT=xts[:, c, m * 128:(m + 1) * 128],
                                 rhs=wvs[:, c, :],
                                 start=(c == 0), stop=(c == NCH - 1))
            nc.vector.tensor_copy(vaug[:, m, :, 0:hd],
                                  pv.rearrange("p (h d) -> p h d", d=hd))
        for t in range(ND):
            for j in range(nseq // FB):
                pq = ppool.tile([128, FB], F32, tag="pq")
                pk = ppool.tile([128, FB], F32, tag="pk")
                for c in range(NCH):
                    nc.tensor.matmul(pq, lhsT=wqs[:, c, t * 128:(t + 1) * 128],
                                     rhs=xts[:, c, j * FB:(j + 1) * FB],
                                     start=(c == 0), stop=(c == NCH - 1))
                for c in range(NCH):
                    nc.tensor.matmul(pk, lhsT=wks[:, c, t * 128:(t + 1) * 128],
                                     rhs=xts[:, c, j * FB:(j + 1) * FB],
                                     start=(c == 0), stop=(c == NCH - 1))
                nc.vector.tensor_copy(qts[:, t, j * FB:(j + 1) * FB], pq)
                nc.vector.tensor_copy(kts[:, t, j * FB:(j + 1) * FB], pk)

    # ---- attention (head pairs A/B, exp batched over jb-wide n1 blocks) ----
    with tc.tile_pool(name="atp", bufs=1, space="PSUM") as atp, \
         tc.tile_pool(name="opp", bufs=1, space="PSUM") as opp, \
         tc.tile_pool(name="apool", bufs=3) as apool, \
         tc.tile_pool(name="ospool", bufs=2) as ospool, \
         tc.tile_pool(name="rpool", bufs=2) as rpool:
        for t in range(ND):
            for jj in range(NJ):
                oA = opp.tile([hd + 1, jb], F32, tag="oA")
                oB = opp.tile([hd + 1, jb], F32, tag="oB")
                for i in range(NN):
                    stA = atp.tile([128, jb], F32, tag="stA")
                    stB = atp.tile([128, jb], F32, tag="stB")
                    for u in range(NU):
                        fs = slice(u * FB, (u + 1) * FB)
                        qs = slice(jj * jb + u * FB, jj * jb + (u + 1) * FB)
                        nc.tensor.matmul(
                            stA[:, fs], lhsT=kts[0:hd, t, i * 128:(i + 1) * 128],
                            rhs=qts[0:hd, t, qs], start=True, stop=True)
                        nc.tensor.matmul(
                            stB[:, fs], lhsT=kts[hd:2 * hd, t, i * 128:(i + 1) * 128],
                            rhs=qts[hd:2 * hd, t, qs], start=True, stop=True)
                    aA = apool.tile([128, jb], BF16, tag="aA")
                    aB = apool.tile([128, jb], BF16, tag="aB")
                    nc.scalar.activation(aA, stA, EXP, scale=SCALE)
                    nc.scalar.activation(aB, stB, EXP, scale=SCALE)
                    for u in range(NU):
                        fs = slice(u * FB, (u + 1) * FB)
                        nc.tensor.matmul(oA[:, fs], lhsT=vaug[:, i, 2 * t, :],
                                         rhs=aA[:, fs],
                                         start=(i == 0), stop=(i == NN - 1))
                        nc.tensor.matmul(oB[:, fs], lhsT=vaug[:, i, 2 * t + 1, :],
                                         rhs=aB[:, fs],
                                         start=(i == 0), stop=(i == NN - 1))
                # drain PSUM quickly, then normalize out of SBUF
                oAs = ospool.tile([hd + 1, jb], F32, tag="oAs")
                oBs = ospool.tile([hd + 1, jb], F32, tag="oBs")
                nc.vector.tensor_copy(oAs, oA)
                nc.vector.tensor_copy(oBs, oB)
                js = slice(jj * jb, (jj + 1) * jb)
                for b, os_ in ((0, oAs), (1, oBs)):
                    r = rpool.tile([1, jb], F32, tag=f"r{b}")
                    R = rpool.tile([hd, jb], F32, tag=f"R{b}")
                    nc.vector.reciprocal(r, os_[hd:hd + 1, :])
                    nc.gpsimd.partition_broadcast(R, r)
                    nc.vector.tensor_mul(ot[b * hd:(b + 1) * hd, t, js],
                                         os_[0:hd, :], R)

    # ---- output projection ----
    with tc.tile_pool(name="ypp", bufs=2, space="PSUM") as ypp, \
         tc.tile_pool(name="ysbp", bufs=3) as ysbp:
        for m in range(NN):
            ys = ysbp.tile([128, cout], y.dtype, tag="ys")
            for cb in range(NCB):
                py = ypp.tile([128, FBO], F32, tag="py")
                for t in range(ND):
                    nc.tensor.matmul(py, lhsT=ot[:, t, m * 128:(m + 1) * 128],
                                     rhs=wos[:, t, cb * FBO:(cb + 1) * FBO],
                                     start=(t == 0), stop=(t == ND - 1))
                nc.vector.tensor_copy(ys[:, cb * FBO:(cb + 1) * FBO], py)
            nc.sync.dma_start(out=y[m * 128:(m + 1) * 128, :], in_=ys)


def _build(nc, xt, wq, wk, wv, wo):
    y = nc.dram_tensor("y", [N, C], BF16, kind="ExternalOutput")
    with tile.TileContext(nc) as tc:
        with ExitStack() as ctx:
            _mhsa_body(ctx, tc, xt[:], wq[:], wk[:], wv[:], wo[:], y[:])
    return y


def _get_compiled():
    if "fn" in _cache:
        return _cache["fn"], _cache["mesh"]
    devs = jax.devices()[:NCORES]
    mesh = Mesh(np.asarray(devs), ("core",))
    sh = NamedSharding(mesh, P("core"))
    bf = ml_dtypes.bfloat16
    structs = tuple(
        jax.ShapeDtypeStruct((NCORES * d0, d1), bf, sharding=sh)
        for d0, d1 in ((C, N), (C, DPG), (C, DPG), (C, DPG), (DPG, C))
    )

    def compile_fn():
        bj = bass_jit(_build)
        f = jax.jit(
            shard_map(
                lambda *a: bj(*a),
                mesh=mesh,
                in_specs=(P("core"),) * 5,
                out_specs=P("core"),
                check_rep=False,
            )
        )
        return f.lower(*structs).compile()

    fn = fast_dispatch_compile(compile_fn)
    _cache["fn"] = fn
    _cache["mesh"] = mesh
    _cache["sharding"] = sh
    return fn, mesh


def _device_put(shards):
    """Place shard arrays with the correct core sharding (avoids a costly
    per-call reshard from device 0)."""
    _get_compiled()
    sh = _cache["sharding"]
    return tuple(jax.device_put(s, sh) for s in shards)


def _make_shards(x, w_qkv, w_out):
    """Per-core inputs, concatenated along axis 0 (core c -> b=c//2, t=c%2)."""
    bf = ml_dtypes.bfloat16
    w_q, w_k, w_v = w_qkv[0:C], w_qkv[C:2 * C], w_qkv[2 * C:3 * C]
    xts, wqs, wks, wvs, wos = [], [], [], [], []
    for core in range(NCORES):
        b, t = divmod(core, TP)
        sl = slice(t * DPG, (t + 1) * DPG)
        xts.append(np.ascontiguousarray(x[b].T).astype(bf))          # [C, N]
        wqs.append(np.ascontiguousarray(w_q[sl].T).astype(bf))       # [C, DPG]
        wks.append(np.ascontiguousarray(w_k[sl].T).astype(bf))
        wvs.append(np.ascontiguousarray(w_v[sl].T).astype(bf))
        wos.append(np.ascontiguousarray(w_out[:, sl].T).astype(bf))  # [DPG, C]
    return (
        np.concatenate(xts, axis=0),
        np.concatenate(wqs, axis=0),
        np.concatenate(wks, axis=0),
        np.concatenate(wvs, axis=0),
        np.concatenate(wos, axis=0),
    )


def kernel(x, w_qkv, w_out, b_out):
    x = np.asarray(x, dtype=np.float32)
    w_qkv = np.asarray(w_qkv, dtype=np.float32)
    w_out = np.asarray(w_out, dtype=np.float32)
    b_out = np.asarray(b_out, dtype=np.float32)

    fn, _ = _get_compiled()
    shards = _device_put(_make_shards(x, w_qkv, w_out))
    out = np.asarray(jax.block_until_ready(fn(*shards)))   # [8*N, C]
    parts = out.reshape(NCORES, N, C).astype(np.float32)
    res = np.empty((B, N, C), dtype=np.float32)
    for b in range(B):
        res[b] = parts[TP * b] + parts[TP * b + 1] + b_out[None, :]
    return res


if __name__ == "__main__":
    rng = np.random.default_rng(0)
    x = rng.standard_normal((B, N, C), dtype=np.float32)
    w_qkv = rng.standard_normal((3 * C, C), dtype=np.float32) * C ** -0.5
    w_out = rng.standard_normal((C, C), dtype=np.float32) * C ** -0.5
    b_out = rng.standard_normal(C, dtype=np.float32) * 0.01
    o = kernel(x=x, w_qkv=w_qkv, w_out=w_out, b_out=b_out)
    print("kernel ran, out shape", o.shape)
